# revision 28
# baseline (speedup 1.0000x reference)
"""Trainium2 Bass kernel for nn_ContrastiveLoss (wav2vec2-style contrastive loss).

Shapes (hardcoded): B=8, C=256, T=1024, M=512 masked positions, K=100 negatives.
Sharding: pure data parallel - batch row b -> NeuronCore b (8 cores).

Strategy: negatives are uploaded as bf16 with each length-256 c-vector padded
to 258 (two zero pad slots). A runtime-registered custom DVE op streams a
[128, kch*258] tile once and emits, per element, an alternating pair of
running prefix sums: cumsum(neg*ctx) at even positions, cumsum(neg^2) at odd
positions (fp32 internal). The two pad slots at the end of each chunk hold the
chunk-complete cumsums of both quantities; a strided copy plus two subtracts
(on GpSimd, off the critical engine) recover per-k dots and sums of squares.
This computes BOTH reductions at ~1.008 DVE cycles per streamed element.

VectorE runs only the scan ops (+ tiny logit math); ScalarE does sqrt/exp/ln;
GpSimd does extraction/clamps. Small DMAs ride the ACT HWDGE queue so the SP
queue only carries the bulk negative stream. Group 0 starts with small tiles
so the first scan begins as early as possible; per-group epilogues are
interleaved so only group 3's tail is serialized. The device returns per-row
losses [128, 4] per core; the host sums and divides.
"""

import numpy as np

TEMP = 0.1
EPS = 1e-8
B, C, T = 8, 256, 1024
M = 512  # masked positions per batch row
K = 100  # negatives per masked position
P = 128  # partitions
G = M // P  # m-groups per core (4)
NPAD = C + 2  # padded chunk length (256 data + 2 pad)

# per-group tile splits (k0, kch); group 0 ramps up for an early first scan,
# growth rate matched to DMA vs scan speed (~1.25x)
_R0 = [4, 5, 6, 8, 10, 12, 15, 20, 20]
_TILES0 = list(zip(np.cumsum([0] + _R0[:-1]).tolist(), _R0))
_RN = [34, 33, 33]
_TILESN = list(zip(np.cumsum([0] + _RN[:-1]).tolist(), _RN))
KCHMAX = max(max(_RN), max(_R0))

_NC = None
_OP = None

OP_NAME = "DUAL_CUMSUM_ANT"


def _register_op():
    """Register the dual-cumsum custom DVE op (idempotent)."""
    global _OP
    if _OP is not None:
        return _OP
    import concourse.dve_ops as dvo
    from concourse.dve_ops import DveOp
    from concourse.dve_spec import Spec, Src0, Src1, Zero, One, select, scan, AluOp, lower
    from concourse.dve_uop import DveOpSpec
    from concourse.dve_table_gen import dve_ver_for

    if OP_NAME in dvo._SUB_OPCODE_FOR_NAME:
        _OP = next(o for o in dvo.OPS if o.name == OP_NAME)
        return _OP

    def _ref(in0, in1, c0, c1, c2):
        Pp = in0.shape[0]
        a = np.asarray(in0, np.float32).reshape(Pp, -1)
        b = np.asarray(in1, np.float32).reshape(Pp, -1)
        prod = np.cumsum(a * b, axis=1)
        sqs = np.cumsum(a * a, axis=1)
        k = np.arange(a.shape[1])
        alt = (k % 2 == 0)  # xor-scan of ones seeded 0: TRUE at even positions
        return np.where(alt[None, :], prod, sqs).reshape(in0.shape)

    s1 = scan(AluOp.ADD, Src0 * Src1)
    s2 = scan(AluOp.ADD, Src0 * Src0)
    alt = scan(AluOp.LOGICAL_XOR, One, init=Zero)
    spec = Spec(body=select(alt, s1, s2), reference=_ref)

    row = max(dvo._SUB_OPCODE_FOR_NAME.values()) + 1
    assert row < 0x20
    dvo._SUB_OPCODE_FOR_NAME[OP_NAME] = row
    ver = dve_ver_for("TRN2")
    uops = lower(spec, ver=ver)
    sha = DveOpSpec(name=OP_NAME, opcode=row, uops=uops, rd1_en=True).sha(ver)
    op = DveOp(OP_NAME, spec, subdim=False, uops_sha={ver: sha})
    dvo.OPS.append(op)
    dvo.CUSTOM_DVE_SPECS[OP_NAME] = spec
    _OP = op
    return op


def _build_nc():
    import concourse.bacc as bacc
    import concourse.tile as tile
    from concourse import mybir

    op = _register_op()

    f32 = mybir.dt.float32
    bf16 = mybir.dt.bfloat16
    Alu = mybir.AluOpType
    Act = mybir.ActivationFunctionType
    EPS2 = EPS * EPS

    nc = bacc.Bacc(trn_type="TRN2")
    neg = nc.dram_tensor("neg", [M, K, NPAD], bf16, kind="ExternalInput")
    ctxp = nc.dram_tensor("ctxp", [M, NPAD], bf16, kind="ExternalInput")
    posg = nc.dram_tensor("posg", [M, C], bf16, kind="ExternalInput")
    rowloss = nc.dram_tensor("rowloss", [P, G], f32, kind="ExternalOutput")

    with tile.TileContext(nc) as tc:
        with (
            tc.tile_pool(name="stream", bufs=4) as stream,
            tc.tile_pool(name="bigp", bufs=2) as bigp,
            tc.tile_pool(name="grp", bufs=G) as grp,
            tc.tile_pool(name="pg", bufs=G) as pg,
            tc.tile_pool(name="scrp", bufs=2) as scrp,
            tc.tile_pool(name="outp", bufs=1) as outp,
        ):
            out_t = outp.tile([P, G], f32, tag="out_t")
            cps_t = outp.tile([P, 2 * G], f32, tag="cps_t")  # css col g, pss col G+g
            cpd_t = outp.tile([P, G], f32, tag="cpd_t")
            l0_t = outp.tile([P, G], f32, tag="l0_t")
            se_t = outp.tile([P, G], f32, tag="se_t")
            lnse_t = outp.tile([P, G], f32, tag="lnse_t")
            t1_t = outp.tile([P, G], f32, tag="t1_t")
            cumt0 = outp.tile([P, KCHMAX + 1, 2], f32, tag="cumt0")
            cumt1 = outp.tile([P, KCHMAX + 1, 2], f32, tag="cumt1")
            cumts = [cumt0, cumt1]
            nc.gpsimd.memset(cumt0[:, 0:1, :], 0.0)
            nc.gpsimd.memset(cumt1[:, 0:1, :], 0.0)
            biasc = outp.tile([P, 1], f32, tag="biasc")
            nc.gpsimd.memset(biasc[:], -1.0 / TEMP)

            gt = {}
            for g in range(G):
                gt[g] = dict(
                    # dn[:, k, 0] = dot_k, dn[:, k, 1] = sumsq_k (interleaved
                    # so one subtract recovers both from the cumsum pairs)
                    dn=pg.tile([P, K, 2], f32, tag="dn", name=f"dn{g}"),
                    logits=pg.tile([P, K + 1], f32, tag="logits", name=f"logits{g}"),
                    nrn=pg.tile([P, K], f32, tag="nrn", name=f"nrn{g}"),
                )

            # --- up-front small DMAs (ACT HWDGE queue) + ScalarE prologue ---
            ctx_ts, pos_ts = {}, {}
            for g in range(G):
                m0 = g * P
                ctx_ts[g] = grp.tile([P, NPAD], bf16, tag="ctx", name=f"ctx{g}")
                # group 0's ctx rides the SP queue ahead of the first neg
                # tile so the first scan's inputs land as early as possible
                eng = nc.sync if g == 0 else nc.scalar
                eng.dma_start(out=ctx_ts[g][:], in_=ctxp[m0 : m0 + P, :])
                pos_ts[g] = grp.tile([P, C], bf16, tag="pos", name=f"pos{g}")
                nc.scalar.dma_start(out=pos_ts[g][:], in_=posg[m0 : m0 + P, :])
            for g in range(G):
                sq_s = scrp.tile([P, C], f32, tag="sq_s")
                nc.scalar.activation(
                    out=sq_s[:], in_=ctx_ts[g][:, 0:C], func=Act.Square,
                    accum_out=cps_t[:, g : g + 1],
                )
                sq_s2 = scrp.tile([P, C], f32, tag="sq_s2")
                nc.scalar.activation(
                    out=sq_s2[:], in_=pos_ts[g][:], func=Act.Square,
                    accum_out=cps_t[:, G + g : G + g + 1],
                )

            def prologue_dve():
                # issued after the first scans so it never delays scan 0;
                # results are first needed by epilogue(0), much later
                for g in range(G):
                    scr = scrp.tile([P, C], bf16, tag="scr")
                    nc.vector.scalar_tensor_tensor(
                        out=scr[:], in0=ctx_ts[g][:, 0:C], scalar=1.0,
                        in1=pos_ts[g][:], op0=Alu.mult, op1=Alu.mult,
                        accum_out=cpd_t[:, g : g + 1],
                    )
                # crn/prn = 1/max(sqrt(ss), EPS) = 1/sqrt(max(ss, EPS^2))
                nc.vector.tensor_scalar_max(cps_t[:], cps_t[:], EPS2)
                nc.scalar.sqrt(cps_t[:], cps_t[:])
                nc.vector.reciprocal_approx_fast(cps_t[:], cps_t[:])

            def pre_epilogue(g):
                # clamp + sqrt issued a scan earlier than the reciprocal so
                # the ACT sqrt latency hides under the next scan
                d = gt[g]
                nc.vector.tensor_scalar_max(d["dn"][:, :, 1], d["dn"][:, :, 1], EPS2)
                nc.scalar.sqrt(d["dn"][:, :, 1], d["dn"][:, :, 1])

            def epilogue(g):
                d = gt[g]
                if g == G - 1:
                    # prewarm the Exp ACT table (runs while DVE computes the
                    # logits below) so the tail exp skips its table load
                    dum2 = scrp.tile([P, 1], f32, tag="dum2")
                    nc.scalar.activation(out=dum2[:], in_=se_t[:, 0:1], func=Act.Exp)
                nc.vector.reciprocal_approx_fast(d["nrn"][:], d["dn"][:, :, 1])
                nc.vector.scalar_tensor_tensor(
                    out=d["logits"][:, 0:1], in0=cpd_t[:, g : g + 1],
                    scalar=cps_t[:, g : g + 1], in1=cps_t[:, G + g : G + g + 1],
                    op0=Alu.mult, op1=Alu.mult,
                )
                nc.vector.tensor_copy(l0_t[:, g : g + 1], d["logits"][:, 0:1])
                nc.vector.scalar_tensor_tensor(
                    out=d["logits"][:, 1 : K + 1], in0=d["dn"][:, :, 0],
                    scalar=cps_t[:, g : g + 1], in1=d["nrn"][:],
                    op0=Alu.mult, op1=Alu.mult,
                )
                # |cosine| <= 1 so logits/TEMP <= 10: a constant shift of -10
                # replaces the per-row max (logsumexp is shift-invariant)
                esc = scrp.tile([P, K + 1], f32, tag="esc")
                nc.scalar.activation(
                    out=esc[:], in_=d["logits"][:], func=Act.Exp,
                    scale=1.0 / TEMP, bias=biasc[:],
                    accum_out=se_t[:, g : g + 1],
                )
                if g == G - 2:
                    # prewarm the Sqrt ACT table so group G-1's tail chain
                    # skips one table load
                    dum = scrp.tile([P, 1], f32, tag="dum")
                    nc.scalar.sqrt(dum[:], se_t[:, 0:1])

            # --- main streaming loop ---
            # subs for tile t are issued after scan t+1 so the ACT-side
            # extraction copy overlaps the next scan instead of stalling DVE;
            # epilogue(g-1) is deferred into group g
            pending_subs = []
            tile_no = 0

            def flush_subs():
                while pending_subs:
                    pending_subs.pop(0)()

            for g in range(G):
                m0 = g * P
                d = gt[g]
                ctx_t = ctx_ts[g]
                tiles = _TILES0 if g == 0 else _TILESN
                for i, (k0, kch) in enumerate(tiles):
                    nt = stream.tile([P, kch, NPAD], bf16, tag="nt")
                    nc.sync.dma_start(
                        out=nt[:], in_=neg[m0 : m0 + P, k0 : k0 + kch, :]
                    )
                    big = bigp.tile([P, kch * NPAD], f32, tag="big")
                    ctx_bc = ctx_t[:].unsqueeze(1).broadcast_to([P, kch, NPAD])
                    nc.vector._custom_dve(
                        op, out=big[:], in0=nt[:], in1=ctx_bc, s0=0.0, s1=0.0
                    )
                    big3 = big[:].rearrange("p (s n) -> p s n", s=kch)
                    cumt = cumts[tile_no % 2]
                    tile_no += 1
                    nc.scalar.copy(cumt[:, 1 : kch + 1, :], big3[:, :, C : C + 2])

                    def subs(d=d, k0=k0, kch=kch, cumt=cumt):
                        # dot cumsum at even pad slot (C), sq cumsum at C+1:
                        # one subtract recovers both interleaved quantities
                        nc.vector.tensor_sub(
                            d["dn"][:, k0 : k0 + kch, :],
                            cumt[:, 1 : kch + 1, :],
                            cumt[:, 0:kch, :],
                        )

                    flush_subs()
                    pending_subs.append(subs)
                    if g == 0 and i == 2:
                        prologue_dve()
                    if g > 0:
                        if i == 0:
                            pre_epilogue(g - 1)
                        elif i == 1:
                            epilogue(g - 1)
            flush_subs()
            pre_epilogue(G - 1)
            epilogue(G - 1)

            # --- tail: loss = (1/TEMP + ln(se)) - l0/TEMP ---
            nc.scalar.activation(out=lnse_t[:], in_=se_t[:], func=Act.Ln)
            nc.vector.tensor_scalar_add(t1_t[:], lnse_t[:], 1.0 / TEMP)
            nc.vector.scalar_tensor_tensor(
                out=out_t[:], in0=l0_t[:], scalar=-1.0 / TEMP, in1=t1_t[:],
                op0=Alu.mult, op1=Alu.add,
            )
            nc.scalar.dma_start(out=rowloss[:], in_=out_t[:])
    nc.finalize()
    return nc


def _get_nc():
    global _NC
    if _NC is None:
        _NC = _build_nc()
    return _NC


def make_in_maps(context, positive, negatives, mask_indices):
    import ml_dtypes

    bf = ml_dtypes.bfloat16
    context = np.asarray(context, dtype=np.float32)
    positive = np.asarray(positive, dtype=np.float32)
    negatives = np.asarray(negatives, dtype=np.float32)
    mask = np.asarray(mask_indices).astype(bool)

    in_maps = []
    for b in range(B):
        idx = np.flatnonzero(mask[b])
        assert idx.size == M, f"row {b}: expected {M} masked, got {idx.size}"
        ctxg = context[b].T[idx]  # [M, C] f32
        posg = positive[b].T[idx]
        ctxp = np.zeros((M, NPAD), dtype=bf)
        ctxp[:, :C] = ctxg.astype(bf)
        negp = np.zeros((M, K, NPAD), dtype=bf)
        negp[:, :, :C] = negatives[b].astype(bf)
        in_maps.append(
            {
                "neg": negp,
                "ctxp": ctxp,
                "posg": np.ascontiguousarray(posg.astype(bf)),
            }
        )
    return in_maps


def kernel(context, positive, negatives, mask_indices, num_masked):
    from concourse.bass_utils import run_bass_kernel_spmd

    nm = int(np.asarray(num_masked))
    assert nm == M, f"kernel hardcodes num_masked={M}, got {nm}"
    assert np.asarray(context).shape == (B, C, T)
    assert np.asarray(negatives).shape == (B, M, K, C)

    in_maps = make_in_maps(context, positive, negatives, mask_indices)
    res = run_bass_kernel_spmd(_get_nc(), in_maps, core_ids=list(range(B)))
    total = np.float64(0.0)
    for r in res.results:
        total += r["rowloss"].astype(np.float64).sum()
    return np.float32(total / (B * M))


# revision 30
# speedup vs baseline: 1.0117x; 1.0117x over previous
"""Trainium2 Bass kernel for nn_ContrastiveLoss (wav2vec2-style contrastive loss).

Shapes (hardcoded): B=8, C=256, T=1024, M=512 masked positions, K=100 negatives.
Sharding: pure data parallel - batch row b -> NeuronCore b (8 cores).

Strategy: negatives are uploaded as bf16 with each length-256 c-vector padded
to 258 (two zero pad slots). A runtime-registered custom DVE op streams a
[128, kch*258] tile once and emits, per element, an alternating pair of
running prefix sums: cumsum(neg*ctx) at even positions, cumsum(neg^2) at odd
positions (fp32 internal). The two pad slots at the end of each chunk hold the
chunk-complete cumsums of both quantities; a strided copy plus two subtracts
(on GpSimd, off the critical engine) recover per-k dots and sums of squares.
This computes BOTH reductions at ~1.008 DVE cycles per streamed element.

VectorE runs only the scan ops (+ tiny logit math); ScalarE does sqrt/exp/ln;
GpSimd does extraction/clamps. Small DMAs ride the ACT HWDGE queue so the SP
queue only carries the bulk negative stream. Group 0 starts with small tiles
so the first scan begins as early as possible; per-group epilogues are
interleaved so only group 3's tail is serialized. The device returns per-row
losses [128, 4] per core; the host sums and divides.
"""

import numpy as np

TEMP = 0.1
EPS = 1e-8
B, C, T = 8, 256, 1024
M = 512  # masked positions per batch row
K = 100  # negatives per masked position
P = 128  # partitions
G = M // P  # m-groups per core (4)
NPAD = C + 2  # padded chunk length (256 data + 2 pad)

# per-group tile splits (k0, kch); group 0 ramps up for an early first scan,
# growth rate matched to DMA vs scan speed (~1.25x)
_R0 = [4, 5, 6, 8, 10, 12, 15, 20, 20]
_TILES0 = list(zip(np.cumsum([0] + _R0[:-1]).tolist(), _R0))
_RN = [25, 25, 25, 25]
_TILESN = list(zip(np.cumsum([0] + _RN[:-1]).tolist(), _RN))
KCHMAX = max(max(_RN), max(_R0))

_NC = None
_OP = None

OP_NAME = "DUAL_CUMSUM_ANT"


def _register_op():
    """Register the dual-cumsum custom DVE op (idempotent)."""
    global _OP
    if _OP is not None:
        return _OP
    import concourse.dve_ops as dvo
    from concourse.dve_ops import DveOp
    from concourse.dve_spec import Spec, Src0, Src1, Zero, One, select, scan, AluOp, lower
    from concourse.dve_uop import DveOpSpec
    from concourse.dve_table_gen import dve_ver_for

    if OP_NAME in dvo._SUB_OPCODE_FOR_NAME:
        _OP = next(o for o in dvo.OPS if o.name == OP_NAME)
        return _OP

    def _ref(in0, in1, c0, c1, c2):
        Pp = in0.shape[0]
        a = np.asarray(in0, np.float32).reshape(Pp, -1)
        b = np.asarray(in1, np.float32).reshape(Pp, -1)
        prod = np.cumsum(a * b, axis=1)
        sqs = np.cumsum(a * a, axis=1)
        k = np.arange(a.shape[1])
        alt = (k % 2 == 0)  # xor-scan of ones seeded 0: TRUE at even positions
        return np.where(alt[None, :], prod, sqs).reshape(in0.shape)

    s1 = scan(AluOp.ADD, Src0 * Src1)
    s2 = scan(AluOp.ADD, Src0 * Src0)
    alt = scan(AluOp.LOGICAL_XOR, One, init=Zero)
    spec = Spec(body=select(alt, s1, s2), reference=_ref)

    row = max(dvo._SUB_OPCODE_FOR_NAME.values()) + 1
    assert row < 0x20
    dvo._SUB_OPCODE_FOR_NAME[OP_NAME] = row
    ver = dve_ver_for("TRN2")
    uops = lower(spec, ver=ver)
    sha = DveOpSpec(name=OP_NAME, opcode=row, uops=uops, rd1_en=True).sha(ver)
    op = DveOp(OP_NAME, spec, subdim=False, uops_sha={ver: sha})
    dvo.OPS.append(op)
    dvo.CUSTOM_DVE_SPECS[OP_NAME] = spec
    _OP = op
    return op


def _build_nc():
    import concourse.bacc as bacc
    import concourse.tile as tile
    from concourse import mybir

    op = _register_op()

    f32 = mybir.dt.float32
    bf16 = mybir.dt.bfloat16
    Alu = mybir.AluOpType
    Act = mybir.ActivationFunctionType
    EPS2 = EPS * EPS

    nc = bacc.Bacc(trn_type="TRN2")
    neg = nc.dram_tensor("neg", [M, K, NPAD], bf16, kind="ExternalInput")
    ctxp = nc.dram_tensor("ctxp", [M, NPAD], bf16, kind="ExternalInput")
    posg = nc.dram_tensor("posg", [M, C], bf16, kind="ExternalInput")
    rowloss = nc.dram_tensor("rowloss", [P, G], f32, kind="ExternalOutput")

    with tile.TileContext(nc) as tc:
        with (
            tc.tile_pool(name="stream", bufs=4) as stream,
            tc.tile_pool(name="bigp", bufs=3) as bigp,
            tc.tile_pool(name="grp", bufs=G) as grp,
            tc.tile_pool(name="pg", bufs=G) as pg,
            tc.tile_pool(name="scrp", bufs=2) as scrp,
            tc.tile_pool(name="outp", bufs=1) as outp,
        ):
            out_t = outp.tile([P, G], f32, tag="out_t")
            cps_t = outp.tile([P, 2 * G], f32, tag="cps_t")  # css col g, pss col G+g
            cpd_t = outp.tile([P, G], f32, tag="cpd_t")
            l0_t = outp.tile([P, G], f32, tag="l0_t")
            se_t = outp.tile([P, G], f32, tag="se_t")
            lnse_t = outp.tile([P, G], f32, tag="lnse_t")
            t1_t = outp.tile([P, G], f32, tag="t1_t")
            cumt0 = outp.tile([P, KCHMAX + 1, 2], f32, tag="cumt0")
            cumt1 = outp.tile([P, KCHMAX + 1, 2], f32, tag="cumt1")
            cumts = [cumt0, cumt1]
            nc.gpsimd.memset(cumt0[:, 0:1, :], 0.0)
            nc.gpsimd.memset(cumt1[:, 0:1, :], 0.0)
            biasc = outp.tile([P, 1], f32, tag="biasc")
            nc.gpsimd.memset(biasc[:], -1.0 / TEMP)

            gt = {}
            for g in range(G):
                gt[g] = dict(
                    # dn[:, k, 0] = dot_k, dn[:, k, 1] = sumsq_k (interleaved
                    # so one subtract recovers both from the cumsum pairs)
                    dn=pg.tile([P, K, 2], f32, tag="dn", name=f"dn{g}"),
                    logits=pg.tile([P, K + 1], f32, tag="logits", name=f"logits{g}"),
                    nrn=pg.tile([P, K], f32, tag="nrn", name=f"nrn{g}"),
                )

            # --- up-front small DMAs (ACT HWDGE queue) + ScalarE prologue ---
            ctx_ts, pos_ts = {}, {}
            for g in range(G):
                m0 = g * P
                ctx_ts[g] = grp.tile([P, NPAD], bf16, tag="ctx", name=f"ctx{g}")
                # group 0's ctx rides the SP queue ahead of the first neg
                # tile so the first scan's inputs land as early as possible
                eng = nc.sync if g == 0 else nc.scalar
                eng.dma_start(out=ctx_ts[g][:], in_=ctxp[m0 : m0 + P, :])
                pos_ts[g] = grp.tile([P, C], bf16, tag="pos", name=f"pos{g}")
                nc.scalar.dma_start(out=pos_ts[g][:], in_=posg[m0 : m0 + P, :])
            for g in range(G):
                sq_s = scrp.tile([P, C], f32, tag="sq_s")
                nc.scalar.activation(
                    out=sq_s[:], in_=ctx_ts[g][:, 0:C], func=Act.Square,
                    accum_out=cps_t[:, g : g + 1],
                )
                sq_s2 = scrp.tile([P, C], f32, tag="sq_s2")
                nc.scalar.activation(
                    out=sq_s2[:], in_=pos_ts[g][:], func=Act.Square,
                    accum_out=cps_t[:, G + g : G + g + 1],
                )

            def prologue_dve():
                # issued after the first scans so it never delays scan 0;
                # results are first needed by epilogue(0), much later
                for g in range(G):
                    scr = scrp.tile([P, C], bf16, tag="scr")
                    nc.vector.scalar_tensor_tensor(
                        out=scr[:], in0=ctx_ts[g][:, 0:C], scalar=1.0,
                        in1=pos_ts[g][:], op0=Alu.mult, op1=Alu.mult,
                        accum_out=cpd_t[:, g : g + 1],
                    )
                # crn/prn = 1/max(sqrt(ss), EPS) = 1/sqrt(max(ss, EPS^2))
                nc.vector.tensor_scalar_max(cps_t[:], cps_t[:], EPS2)
                nc.scalar.sqrt(cps_t[:], cps_t[:])
                nc.vector.reciprocal_approx_fast(cps_t[:], cps_t[:])

            def pre_epilogue(g):
                # clamp + sqrt issued a scan earlier than the reciprocal so
                # the ACT sqrt latency hides under the next scan
                d = gt[g]
                nc.vector.tensor_scalar_max(d["dn"][:, :, 1], d["dn"][:, :, 1], EPS2)
                nc.scalar.sqrt(d["dn"][:, :, 1], d["dn"][:, :, 1])

            def epilogue(g):
                d = gt[g]
                if g == G - 1:
                    # prewarm the Exp ACT table (runs while DVE computes the
                    # logits below) so the tail exp skips its table load
                    dum2 = scrp.tile([P, 1], f32, tag="dum2")
                    nc.scalar.activation(out=dum2[:], in_=se_t[:, 0:1], func=Act.Exp)
                nc.vector.reciprocal_approx_fast(d["nrn"][:], d["dn"][:, :, 1])
                nc.vector.scalar_tensor_tensor(
                    out=d["logits"][:, 0:1], in0=cpd_t[:, g : g + 1],
                    scalar=cps_t[:, g : g + 1], in1=cps_t[:, G + g : G + g + 1],
                    op0=Alu.mult, op1=Alu.mult,
                )
                nc.vector.tensor_copy(l0_t[:, g : g + 1], d["logits"][:, 0:1])
                nc.vector.scalar_tensor_tensor(
                    out=d["logits"][:, 1 : K + 1], in0=d["dn"][:, :, 0],
                    scalar=cps_t[:, g : g + 1], in1=d["nrn"][:],
                    op0=Alu.mult, op1=Alu.mult,
                )
                # |cosine| <= 1 so logits/TEMP <= 10: a constant shift of -10
                # replaces the per-row max (logsumexp is shift-invariant)
                esc = scrp.tile([P, K + 1], f32, tag="esc")
                nc.scalar.activation(
                    out=esc[:], in_=d["logits"][:], func=Act.Exp,
                    scale=1.0 / TEMP, bias=biasc[:],
                    accum_out=se_t[:, g : g + 1],
                )
                if g == G - 2:
                    # prewarm the Sqrt ACT table so group G-1's tail chain
                    # skips one table load
                    dum = scrp.tile([P, 1], f32, tag="dum")
                    nc.scalar.sqrt(dum[:], se_t[:, 0:1])

            # --- main streaming loop ---
            # subs for tile t are issued after scan t+1 so the ACT-side
            # extraction copy overlaps the next scan instead of stalling DVE;
            # epilogue(g-1) is deferred into group g
            pending_subs = []
            tile_no = 0

            def flush_subs():
                while pending_subs:
                    pending_subs.pop(0)()

            for g in range(G):
                m0 = g * P
                d = gt[g]
                ctx_t = ctx_ts[g]
                tiles = _TILES0 if g == 0 else _TILESN
                for i, (k0, kch) in enumerate(tiles):
                    nt = stream.tile([P, kch, NPAD], bf16, tag="nt")
                    nc.sync.dma_start(
                        out=nt[:], in_=neg[m0 : m0 + P, k0 : k0 + kch, :]
                    )
                    big = bigp.tile([P, kch * NPAD], f32, tag="big")
                    ctx_bc = ctx_t[:].unsqueeze(1).broadcast_to([P, kch, NPAD])
                    nc.vector._custom_dve(
                        op, out=big[:], in0=nt[:], in1=ctx_bc, s0=0.0, s1=0.0
                    )
                    big3 = big[:].rearrange("p (s n) -> p s n", s=kch)
                    cumt = cumts[tile_no % 2]
                    tile_no += 1
                    nc.scalar.copy(cumt[:, 1 : kch + 1, :], big3[:, :, C : C + 2])

                    def subs(d=d, k0=k0, kch=kch, cumt=cumt):
                        # dot cumsum at even pad slot (C), sq cumsum at C+1:
                        # one subtract recovers both interleaved quantities
                        nc.vector.tensor_sub(
                            d["dn"][:, k0 : k0 + kch, :],
                            cumt[:, 1 : kch + 1, :],
                            cumt[:, 0:kch, :],
                        )

                    flush_subs()
                    pending_subs.append(subs)
                    if g == 0 and i == 2:
                        prologue_dve()
                    if g > 0:
                        if i == 0:
                            pre_epilogue(g - 1)
                        elif i == 1:
                            epilogue(g - 1)
            flush_subs()
            pre_epilogue(G - 1)
            epilogue(G - 1)

            # --- tail: loss = (1/TEMP + ln(se)) - l0/TEMP ---
            nc.scalar.activation(out=lnse_t[:], in_=se_t[:], func=Act.Ln)
            nc.vector.tensor_scalar_add(t1_t[:], lnse_t[:], 1.0 / TEMP)
            nc.vector.scalar_tensor_tensor(
                out=out_t[:], in0=l0_t[:], scalar=-1.0 / TEMP, in1=t1_t[:],
                op0=Alu.mult, op1=Alu.add,
            )
            nc.scalar.dma_start(out=rowloss[:], in_=out_t[:])
    nc.finalize()
    return nc


def _get_nc():
    global _NC
    if _NC is None:
        _NC = _build_nc()
    return _NC


def make_in_maps(context, positive, negatives, mask_indices):
    import ml_dtypes

    bf = ml_dtypes.bfloat16
    context = np.asarray(context, dtype=np.float32)
    positive = np.asarray(positive, dtype=np.float32)
    negatives = np.asarray(negatives, dtype=np.float32)
    mask = np.asarray(mask_indices).astype(bool)

    in_maps = []
    for b in range(B):
        idx = np.flatnonzero(mask[b])
        assert idx.size == M, f"row {b}: expected {M} masked, got {idx.size}"
        ctxg = context[b].T[idx]  # [M, C] f32
        posg = positive[b].T[idx]
        ctxp = np.zeros((M, NPAD), dtype=bf)
        ctxp[:, :C] = ctxg.astype(bf)
        negp = np.zeros((M, K, NPAD), dtype=bf)
        negp[:, :, :C] = negatives[b].astype(bf)
        in_maps.append(
            {
                "neg": negp,
                "ctxp": ctxp,
                "posg": np.ascontiguousarray(posg.astype(bf)),
            }
        )
    return in_maps


def kernel(context, positive, negatives, mask_indices, num_masked):
    from concourse.bass_utils import run_bass_kernel_spmd

    nm = int(np.asarray(num_masked))
    assert nm == M, f"kernel hardcodes num_masked={M}, got {nm}"
    assert np.asarray(context).shape == (B, C, T)
    assert np.asarray(negatives).shape == (B, M, K, C)

    in_maps = make_in_maps(context, positive, negatives, mask_indices)
    res = run_bass_kernel_spmd(_get_nc(), in_maps, core_ids=list(range(B)))
    total = np.float64(0.0)
    for r in res.results:
        total += r["rowloss"].astype(np.float64).sum()
    return np.float32(total / (B * M))


# revision 31
# speedup vs baseline: 1.0262x; 1.0144x over previous
"""Trainium2 Bass kernel for nn_ContrastiveLoss (wav2vec2-style contrastive loss).

Shapes (hardcoded): B=8, C=256, T=1024, M=512 masked positions, K=100 negatives.
Sharding: pure data parallel - batch row b -> NeuronCore b (8 cores).

Strategy: negatives are uploaded as bf16 with each length-256 c-vector padded
to 258 (two zero pad slots). A runtime-registered custom DVE op streams a
[128, kch*258] tile once and emits, per element, an alternating pair of
running prefix sums: cumsum(neg*ctx) at even positions, cumsum(neg^2) at odd
positions (fp32 internal). The two pad slots at the end of each chunk hold the
chunk-complete cumsums of both quantities; a strided copy plus two subtracts
(on GpSimd, off the critical engine) recover per-k dots and sums of squares.
This computes BOTH reductions at ~1.008 DVE cycles per streamed element.

VectorE runs only the scan ops (+ tiny logit math); ScalarE does sqrt/exp/ln;
GpSimd does extraction/clamps. Small DMAs ride the ACT HWDGE queue so the SP
queue only carries the bulk negative stream. Group 0 starts with small tiles
so the first scan begins as early as possible; per-group epilogues are
interleaved so only group 3's tail is serialized. The device returns per-row
losses [128, 4] per core; the host sums and divides.
"""

import numpy as np

TEMP = 0.1
EPS = 1e-8
B, C, T = 8, 256, 1024
M = 512  # masked positions per batch row
K = 100  # negatives per masked position
P = 128  # partitions
G = M // P  # m-groups per core (4)
NPAD = C + 2  # padded chunk length (256 data + 2 pad)

# per-group tile splits (k0, kch); group 0 ramps up for an early first scan,
# growth rate matched to DMA vs scan speed (~1.25x)
_R0 = [4, 5, 6, 8, 10, 12, 15, 20, 20]
_TILES0 = list(zip(np.cumsum([0] + _R0[:-1]).tolist(), _R0))
_RN = [25, 25, 25, 25]
_TILESN = list(zip(np.cumsum([0] + _RN[:-1]).tolist(), _RN))
KCHMAX = max(max(_RN), max(_R0))

_NC = None
_OP = None

OP_NAME = "DUAL_CUMSUM_ANT"


def _register_op():
    """Register the dual-cumsum custom DVE op (idempotent)."""
    global _OP
    if _OP is not None:
        return _OP
    import concourse.dve_ops as dvo
    from concourse.dve_ops import DveOp
    from concourse.dve_spec import Spec, Src0, Src1, Zero, One, select, scan, AluOp, lower
    from concourse.dve_uop import DveOpSpec
    from concourse.dve_table_gen import dve_ver_for

    if OP_NAME in dvo._SUB_OPCODE_FOR_NAME:
        _OP = next(o for o in dvo.OPS if o.name == OP_NAME)
        return _OP

    def _ref(in0, in1, c0, c1, c2):
        Pp = in0.shape[0]
        a = np.asarray(in0, np.float32).reshape(Pp, -1)
        b = np.asarray(in1, np.float32).reshape(Pp, -1)
        prod = np.cumsum(a * b, axis=1)
        sqs = np.cumsum(a * a, axis=1)
        k = np.arange(a.shape[1])
        alt = (k % 2 == 0)  # xor-scan of ones seeded 0: TRUE at even positions
        return np.where(alt[None, :], prod, sqs).reshape(in0.shape)

    s1 = scan(AluOp.ADD, Src0 * Src1)
    s2 = scan(AluOp.ADD, Src0 * Src0)
    alt = scan(AluOp.LOGICAL_XOR, One, init=Zero)
    spec = Spec(body=select(alt, s1, s2), reference=_ref)

    row = max(dvo._SUB_OPCODE_FOR_NAME.values()) + 1
    assert row < 0x20
    dvo._SUB_OPCODE_FOR_NAME[OP_NAME] = row
    ver = dve_ver_for("TRN2")
    uops = lower(spec, ver=ver)
    sha = DveOpSpec(name=OP_NAME, opcode=row, uops=uops, rd1_en=True).sha(ver)
    op = DveOp(OP_NAME, spec, subdim=False, uops_sha={ver: sha})
    dvo.OPS.append(op)
    dvo.CUSTOM_DVE_SPECS[OP_NAME] = spec
    _OP = op
    return op


def _build_nc():
    import concourse.bacc as bacc
    import concourse.tile as tile
    from concourse import mybir

    op = _register_op()

    f32 = mybir.dt.float32
    bf16 = mybir.dt.bfloat16
    Alu = mybir.AluOpType
    Act = mybir.ActivationFunctionType
    EPS2 = EPS * EPS

    nc = bacc.Bacc(trn_type="TRN2")
    neg = nc.dram_tensor("neg", [M, K, NPAD], bf16, kind="ExternalInput")
    ctxp = nc.dram_tensor("ctxp", [M, NPAD], bf16, kind="ExternalInput")
    posg = nc.dram_tensor("posg", [M, C], bf16, kind="ExternalInput")
    rowloss = nc.dram_tensor("rowloss", [P, 2 * G], f32, kind="ExternalOutput")

    with tile.TileContext(nc) as tc:
        with (
            tc.tile_pool(name="stream", bufs=4) as stream,
            tc.tile_pool(name="bigp", bufs=3) as bigp,
            tc.tile_pool(name="grp", bufs=G) as grp,
            tc.tile_pool(name="pg", bufs=G) as pg,
            tc.tile_pool(name="scrp", bufs=2) as scrp,
            tc.tile_pool(name="outp", bufs=1) as outp,
        ):
            # out_t: exp-sums (cols 0..G-1) and positive sims (cols G..2G-1);
            # the tiny final ln/combine runs on the host with the mean
            out_t = outp.tile([P, 2 * G], f32, tag="out_t")
            cps_t = outp.tile([P, 2 * G], f32, tag="cps_t")  # css col g, pss col G+g
            cpd_t = outp.tile([P, G], f32, tag="cpd_t")
            cumt0 = outp.tile([P, KCHMAX + 1, 2], f32, tag="cumt0")
            cumt1 = outp.tile([P, KCHMAX + 1, 2], f32, tag="cumt1")
            cumts = [cumt0, cumt1]
            nc.gpsimd.memset(cumt0[:, 0:1, :], 0.0)
            nc.gpsimd.memset(cumt1[:, 0:1, :], 0.0)
            biasc = outp.tile([P, 1], f32, tag="biasc")
            nc.gpsimd.memset(biasc[:], -1.0 / TEMP)

            gt = {}
            for g in range(G):
                gt[g] = dict(
                    # dn[:, k, 0] = dot_k, dn[:, k, 1] = sumsq_k (interleaved
                    # so one subtract recovers both from the cumsum pairs)
                    dn=pg.tile([P, K, 2], f32, tag="dn", name=f"dn{g}"),
                    logits=pg.tile([P, K + 1], f32, tag="logits", name=f"logits{g}"),
                    nrn=pg.tile([P, K], f32, tag="nrn", name=f"nrn{g}"),
                )

            # --- up-front small DMAs (ACT HWDGE queue) + ScalarE prologue ---
            ctx_ts, pos_ts = {}, {}
            for g in range(G):
                m0 = g * P
                ctx_ts[g] = grp.tile([P, NPAD], bf16, tag="ctx", name=f"ctx{g}")
                # group 0's ctx rides the SP queue ahead of the first neg
                # tile so the first scan's inputs land as early as possible
                eng = nc.sync if g == 0 else nc.scalar
                eng.dma_start(out=ctx_ts[g][:], in_=ctxp[m0 : m0 + P, :])
                pos_ts[g] = grp.tile([P, C], bf16, tag="pos", name=f"pos{g}")
                nc.scalar.dma_start(out=pos_ts[g][:], in_=posg[m0 : m0 + P, :])
            for g in range(G):
                sq_s = scrp.tile([P, C], f32, tag="sq_s")
                nc.scalar.activation(
                    out=sq_s[:], in_=ctx_ts[g][:, 0:C], func=Act.Square,
                    accum_out=cps_t[:, g : g + 1],
                )
                sq_s2 = scrp.tile([P, C], f32, tag="sq_s2")
                nc.scalar.activation(
                    out=sq_s2[:], in_=pos_ts[g][:], func=Act.Square,
                    accum_out=cps_t[:, G + g : G + g + 1],
                )

            def prologue_dve():
                # issued after the first scans so it never delays scan 0;
                # results are first needed by epilogue(0), much later
                for g in range(G):
                    scr = scrp.tile([P, C], bf16, tag="scr")
                    nc.vector.scalar_tensor_tensor(
                        out=scr[:], in0=ctx_ts[g][:, 0:C], scalar=1.0,
                        in1=pos_ts[g][:], op0=Alu.mult, op1=Alu.mult,
                        accum_out=cpd_t[:, g : g + 1],
                    )
                # crn/prn = 1/max(sqrt(ss), EPS) = 1/sqrt(max(ss, EPS^2))
                nc.vector.tensor_scalar_max(cps_t[:], cps_t[:], EPS2)
                nc.scalar.sqrt(cps_t[:], cps_t[:])
                nc.vector.reciprocal_approx_fast(cps_t[:], cps_t[:])

            def pre_epilogue(g):
                # clamp + sqrt issued a scan earlier than the reciprocal so
                # the ACT sqrt latency hides under the next scan
                d = gt[g]
                nc.vector.tensor_scalar_max(d["dn"][:, :, 1], d["dn"][:, :, 1], EPS2)
                nc.scalar.sqrt(d["dn"][:, :, 1], d["dn"][:, :, 1])

            def epilogue(g):
                d = gt[g]
                if g == G - 1:
                    # prewarm the Exp ACT table (runs while DVE computes the
                    # logits below) so the tail exp skips its table load
                    dum2 = scrp.tile([P, 1], f32, tag="dum2")
                    nc.scalar.activation(out=dum2[:], in_=cps_t[:, 0:1], func=Act.Exp)
                nc.vector.reciprocal_approx_fast(d["nrn"][:], d["dn"][:, :, 1])
                nc.vector.scalar_tensor_tensor(
                    out=d["logits"][:, 0:1], in0=cpd_t[:, g : g + 1],
                    scalar=cps_t[:, g : g + 1], in1=cps_t[:, G + g : G + g + 1],
                    op0=Alu.mult, op1=Alu.mult,
                )
                nc.vector.tensor_copy(out_t[:, G + g : G + g + 1], d["logits"][:, 0:1])
                nc.vector.scalar_tensor_tensor(
                    out=d["logits"][:, 1 : K + 1], in0=d["dn"][:, :, 0],
                    scalar=cps_t[:, g : g + 1], in1=d["nrn"][:],
                    op0=Alu.mult, op1=Alu.mult,
                )
                # |cosine| <= 1 so logits/TEMP <= 10: a constant shift of -10
                # replaces the per-row max (logsumexp is shift-invariant)
                esc = scrp.tile([P, K + 1], f32, tag="esc")
                nc.scalar.activation(
                    out=esc[:], in_=d["logits"][:], func=Act.Exp,
                    scale=1.0 / TEMP, bias=biasc[:],
                    accum_out=out_t[:, g : g + 1],
                )
                if g == G - 2:
                    # prewarm the Sqrt ACT table so group G-1's tail chain
                    # skips one table load
                    dum = scrp.tile([P, 1], f32, tag="dum")
                    nc.scalar.sqrt(dum[:], cps_t[:, 0:1])

            # --- main streaming loop ---
            # subs for tile t are issued after scan t+1 so the ACT-side
            # extraction copy overlaps the next scan instead of stalling DVE;
            # epilogue(g-1) is deferred into group g
            pending_subs = []
            tile_no = 0

            def flush_subs():
                while pending_subs:
                    pending_subs.pop(0)()

            for g in range(G):
                m0 = g * P
                d = gt[g]
                ctx_t = ctx_ts[g]
                tiles = _TILES0 if g == 0 else _TILESN
                for i, (k0, kch) in enumerate(tiles):
                    nt = stream.tile([P, kch, NPAD], bf16, tag="nt")
                    nc.sync.dma_start(
                        out=nt[:], in_=neg[m0 : m0 + P, k0 : k0 + kch, :]
                    )
                    big = bigp.tile([P, kch * NPAD], f32, tag="big")
                    ctx_bc = ctx_t[:].unsqueeze(1).broadcast_to([P, kch, NPAD])
                    nc.vector._custom_dve(
                        op, out=big[:], in0=nt[:], in1=ctx_bc, s0=0.0, s1=0.0
                    )
                    big3 = big[:].rearrange("p (s n) -> p s n", s=kch)
                    cumt = cumts[tile_no % 2]
                    tile_no += 1
                    nc.scalar.copy(cumt[:, 1 : kch + 1, :], big3[:, :, C : C + 2])

                    def subs(d=d, k0=k0, kch=kch, cumt=cumt):
                        # dot cumsum at even pad slot (C), sq cumsum at C+1:
                        # one subtract recovers both interleaved quantities
                        nc.vector.tensor_sub(
                            d["dn"][:, k0 : k0 + kch, :],
                            cumt[:, 1 : kch + 1, :],
                            cumt[:, 0:kch, :],
                        )

                    flush_subs()
                    pending_subs.append(subs)
                    if g == 0 and i == 2:
                        prologue_dve()
                    if g > 0:
                        if i == 0:
                            pre_epilogue(g - 1)
                        elif i == 1:
                            epilogue(g - 1)
            flush_subs()
            pre_epilogue(G - 1)
            epilogue(G - 1)

            # --- tail: ship (se, l0); host computes 1/T + ln(se) - l0/T ---
            nc.scalar.dma_start(out=rowloss[:], in_=out_t[:])
    nc.finalize()
    return nc


def _get_nc():
    global _NC
    if _NC is None:
        _NC = _build_nc()
    return _NC


def make_in_maps(context, positive, negatives, mask_indices):
    import ml_dtypes

    bf = ml_dtypes.bfloat16
    context = np.asarray(context, dtype=np.float32)
    positive = np.asarray(positive, dtype=np.float32)
    negatives = np.asarray(negatives, dtype=np.float32)
    mask = np.asarray(mask_indices).astype(bool)

    in_maps = []
    for b in range(B):
        idx = np.flatnonzero(mask[b])
        assert idx.size == M, f"row {b}: expected {M} masked, got {idx.size}"
        ctxg = context[b].T[idx]  # [M, C] f32
        posg = positive[b].T[idx]
        ctxp = np.zeros((M, NPAD), dtype=bf)
        ctxp[:, :C] = ctxg.astype(bf)
        negp = np.zeros((M, K, NPAD), dtype=bf)
        negp[:, :, :C] = negatives[b].astype(bf)
        in_maps.append(
            {
                "neg": negp,
                "ctxp": ctxp,
                "posg": np.ascontiguousarray(posg.astype(bf)),
            }
        )
    return in_maps


def kernel(context, positive, negatives, mask_indices, num_masked):
    from concourse.bass_utils import run_bass_kernel_spmd

    nm = int(np.asarray(num_masked))
    assert nm == M, f"kernel hardcodes num_masked={M}, got {nm}"
    assert np.asarray(context).shape == (B, C, T)
    assert np.asarray(negatives).shape == (B, M, K, C)

    in_maps = make_in_maps(context, positive, negatives, mask_indices)
    res = run_bass_kernel_spmd(_get_nc(), in_maps, core_ids=list(range(B)))
    total = np.float64(0.0)
    for r in res.results:
        out = r["rowloss"].astype(np.float64)
        se, l0 = out[:, :G], out[:, G:]
        total += (1.0 / TEMP + np.log(se) - l0 / TEMP).sum()
    return np.float32(total / (B * M))
